# revision 7
# baseline (speedup 1.0000x reference)
"""Trainium2 Bass kernel for the pairwise message-passing network action model.

Full (unsharded) inputs in, full output out. Internally shards agents (rows i)
across 8 NeuronCores, data-parallel, per the row-sharding strategy:
each core computes a 128 x 1024 block of the pairwise interaction, does the
masked max-pool locally, and runs the replicated MLP head on its 128 agents.

Math restructure (exactly equivalent to the reference up to fp rounding):
  conv1: h1_ij = relu(W1d s_i - W1d s_j + b1 + delta_ij w1e)
       = relu(A_i - A_j + b1)  with A = s @ W1d.T   (diagonal handled separately)
  mask:  pooled_i = max_j mask_ij * relu(W2 h1_ij + b2)
       = relu(b2 + max_j (W2 h1_ij - pen_ij))  with pen in {0, BIG}
    The penalty row is appended as a 65th contraction row of the second
    matmul (lhsT row 64 = -1), so masking is free on the PE.
  diag:  z_ii = W2 relu(b1 + w1e) is a constant vector; the diagonal is
    excluded via pen and max'd back in at the end.
The mask itself (dist < 2) is computed on the host in fp32 with the same
operation order as the reference, so threshold decisions match bit-exactly.
"""

import os
import sys
import time
import types

import numpy as np
import ml_dtypes

import concourse.bass as bass
import concourse.tile as tile
from concourse import mybir
from concourse.vector_clock import ScopedClock

N = 1024
SD = 10
NCORES = 8
RPC = N // NCORES  # rows (agents) per core
BIG = np.float32(4096.0)
OBS_R2 = np.float32(4.0)  # OBS_RADIUS ** 2
BF16 = ml_dtypes.bfloat16

F32 = mybir.dt.float32
BF = mybir.dt.bfloat16


def _patch_tile_drain():
    """This walrus build rejects >1 sync-wait on CTRL (Drain/Nop) instructions;
    split the Tile exit drain's waits across single-wait nops."""
    if getattr(tile.TileContext, "_drain_split_patched", False):
        return

    def _drain_and_barrier_split(self, tick_clock, wait_clock):
        nc = self.nc
        drain_inst = nc.sync.drain()
        wait_clock.add_sem_waits(
            drain_inst.ins, ScopedClock({None: tick_clock.global_clock})
        )
        si = drain_inst.ins.sync_info
        waits = list(si.on_wait) if si and si.on_wait else []
        if len(waits) > 1:
            si.on_wait = [waits[0]]
            for w in waits[1:]:
                nop = nc.sync.nop(nofuse=True)
                nsi = nop.ins.sync_info
                if nsi is None:
                    nop.ins.sync_info = mybir.SyncInfo(on_wait=[w], on_update=[])
                else:
                    nsi.on_wait = [w]
        nc.all_engine_barrier()
        assert self.sems is not None
        popped = nc._tile_sem_poison_stack.pop()
        assert popped is self._sem_poison
        nc.clear_and_free_semaphores(list(self.sems.allocated().values()))
        nc.all_engine_barrier()

    tile.TileContext._drain_and_barrier = _drain_and_barrier_split

    # The Pool-engine (CoreV2) and CTRL instruction encodings in this build
    # accept only one embedded sync-wait. Hoist extra waits onto same-engine
    # NOPs right before the instruction (engines dispatch in order, so the
    # semantics are unchanged).
    _orig_postorder = tile.postorder_instruction_blocks
    _ws_counter = [0]

    def _needs_split(ins):
        return True

    def _postorder_with_wait_split(obib, start_bb, postordered):
        for bb, insts in obib.items():
            out = []
            changed = False
            for ins in insts:
                si = ins.sync_info
                waits = list(si.on_wait) if si and si.on_wait else []
                if len(waits) > 1 and _needs_split(ins):
                    changed = True
                    for w in waits[1:]:
                        _ws_counter[0] += 1
                        nop = mybir.InstNoOp(
                            name=f"WSPLIT-{_ws_counter[0]}", ins=[], outs=[])
                        nop.engine = ins.engine
                        nop.sync_info = mybir.SyncInfo(on_wait=[w], on_update=[])
                        out.append(nop)
                    si.on_wait = [waits[0]]
                out.append(ins)
            if changed:
                insts[:] = out
        return _orig_postorder(obib, start_bb, postordered)

    tile.postorder_instruction_blocks = _postorder_with_wait_split
    tile.TileContext._drain_split_patched = True


# ---------------------------------------------------------------------------
# device module
# ---------------------------------------------------------------------------

_INPUT_SPECS = [
    # name, shape, dtype
    ("atneg", (64, N), BF),        # -A.T, shared across cores
    ("biasc", (64, RPC), F32),     # (A[I] + b1).T, per core
    ("pen", (RPC, N), BF),         # penalty rows (0 / BIG), per core, diag=BIG
    ("l65", (65, 128), BF),        # [W2.T ; -1s] stationary operand
    ("cdiag", (128, 1), F32),      # W2 @ relu(b1 + w1e)
    ("b2c", (128, 1), F32),
    ("w1at", (128, 64), F32),      # Wf1[:, :128].T
    ("w1bt", (SD, 64), F32),       # Wf1[:, 128:].T
    ("bf1c", (64, 1), F32),
    ("w2t", (64, 128), F32),
    ("bf2c", (128, 1), F32),
    ("w3t", (128, 64), F32),
    ("bf3c", (64, 1), F32),
    ("w4t", (64, SD), F32),
    ("bf4c", (SD, 1), F32),
    ("dst", (SD, RPC), F32),       # (s - s_ref).T block
    ("ureft", (SD, RPC), F32),     # ((s_ref - s)/dt).T block
    ("ident", (SD, SD), F32),
]


def _emit(tc, ins, outs, h1_engine="gpsimd"):
    nc = tc.nc
    AF = mybir.ActivationFunctionType
    ALU = mybir.AluOpType
    GRP = 2  # agents per psum group

    with tc.tile_pool(name="consts", bufs=1) as consts, \
         tc.tile_pool(name="persist", bufs=1) as persist:
        ct = {}
        for name, shape, dt in _INPUT_SPECS:
            t = consts.tile(list(shape), dt, tag=name)
            nc.sync.dma_start(t[:], ins[name][:])
            ct[name] = t

        pooledT = persist.tile([128, RPC], F32, tag="pooledT")

        h1eng = nc.gpsimd if h1_engine == "gpsimd" else nc.vector

        with tc.tile_pool(name="h1", bufs=4) as h1p, \
             tc.tile_pool(name="ps", bufs=2, space="PSUM") as psp:
            for r in range(RPC // GRP):
                ps = psp.tile([128, GRP * N], F32, tag="ps")
                for k in range(GRP):
                    i = GRP * r + k
                    h1t = h1p.tile([65, N], BF, tag=f"h1_{k}")
                    # rows 0:64 = relu(-A_j + (A_i + b1)); row 64 = penalty
                    h1eng.tensor_scalar(
                        h1t[0:64, :], ct["atneg"][:],
                        scalar1=ct["biasc"][:, i:i + 1], scalar2=0.0,
                        op0=ALU.add, op1=ALU.max,
                    )
                    nc.sync.dma_start(h1t[64:65, :], ct["pen"][i:i + 1, :])
                    for b in range(N // 512):
                        nc.tensor.matmul(
                            ps[:, k * N + b * 512: k * N + (b + 1) * 512],
                            ct["l65"][:],
                            h1t[:, b * 512:(b + 1) * 512],
                            start=True, stop=True,
                        )
                # masked max over neighbors for GRP agents in one reduce
                nc.vector.tensor_reduce(
                    pooledT[:, GRP * r: GRP * (r + 1)],
                    ps[:].rearrange("p (g n) -> p g n", g=GRP),
                    axis=mybir.AxisListType.X,
                    op=ALU.max,
                )

        # pooled = relu(max(pooled, cdiag) + b2), then the MLP head
        with tc.tile_pool(name="hsb", bufs=2) as hsb, \
             tc.tile_pool(name="hps", bufs=1, space="PSUM") as hps:
            pmax = hsb.tile([128, RPC], F32, tag="pmax")
            nc.vector.tensor_scalar(
                pmax[:], pooledT[:],
                scalar1=ct["cdiag"][:], scalar2=ct["b2c"][:],
                op0=ALU.max, op1=ALU.add,
            )
            feat = hsb.tile([128, RPC], F32, tag="feat")
            nc.scalar.activation(feat[:], pmax[:], AF.Relu)

            ps1 = hps.tile([64, RPC], F32, tag="ps1")
            nc.tensor.matmul(ps1[:], ct["w1at"][:], feat[:], start=True, stop=False)
            nc.tensor.matmul(ps1[:], ct["w1bt"][:], ct["dst"][:], start=False, stop=True)
            y1 = hsb.tile([64, RPC], F32, tag="y1")
            nc.scalar.activation(y1[:], ps1[:], AF.Relu, bias=ct["bf1c"][:])

            ps2 = hps.tile([128, RPC], F32, tag="ps2")
            nc.tensor.matmul(ps2[:], ct["w2t"][:], y1[:], start=True, stop=True)
            y2 = hsb.tile([128, RPC], F32, tag="y2")
            nc.scalar.activation(y2[:], ps2[:], AF.Relu, bias=ct["bf2c"][:])

            ps3 = hps.tile([64, RPC], F32, tag="ps3")
            nc.tensor.matmul(ps3[:], ct["w3t"][:], y2[:], start=True, stop=True)
            y3 = hsb.tile([64, RPC], F32, tag="y3")
            nc.scalar.activation(y3[:], ps3[:], AF.Relu, bias=ct["bf3c"][:])

            ps4 = hps.tile([SD, RPC], F32, tag="ps4")
            nc.tensor.matmul(ps4[:], ct["w4t"][:], y3[:], start=True, stop=True)
            y4 = hsb.tile([SD, RPC], F32, tag="y4")
            nc.scalar.activation(y4[:], ps4[:], AF.Identity, bias=ct["bf4c"][:])
            outT = hsb.tile([SD, RPC], F32, tag="outT")
            nc.vector.tensor_add(outT[:], y4[:], ct["ureft"][:])

            # transpose (SD, RPC) -> (RPC, SD) on the PE, then store
            pst = hps.tile([RPC, SD], F32, tag="pst")
            nc.tensor.transpose(pst[:], outT[:], ct["ident"][:])
            osb = hsb.tile([RPC, SD], F32, tag="osb")
            nc.scalar.copy(osb[:], pst[:])
            nc.sync.dma_start(outs["yout"][:], osb[:])


def build_module(h1_engine="gpsimd"):
    _patch_tile_drain()
    nc = bass.Bass("TRN2", target_bir_lowering=False, debug=False,
                   num_devices=NCORES)
    ins = {}
    for name, shape, dt in _INPUT_SPECS:
        ins[name] = nc.dram_tensor(name, shape, dt, kind="ExternalInput").ap()
    outs = {"yout": nc.dram_tensor("yout", (RPC, SD), F32,
                                   kind="ExternalOutput").ap()}
    with tile.TileContext(nc) as tc:
        _emit(tc, ins, outs, h1_engine=h1_engine)
    return nc


# ---------------------------------------------------------------------------
# host side
# ---------------------------------------------------------------------------

def prepare_in_maps(inputs):
    s = np.asarray(inputs["s"], np.float32)
    s_ref = np.asarray(inputs["s_ref"], np.float32)
    W1 = np.asarray(inputs["W1"], np.float32)
    b1 = np.asarray(inputs["b1"], np.float32)
    W2 = np.asarray(inputs["W2"], np.float32)
    b2 = np.asarray(inputs["b2"], np.float32)
    Wf1 = np.asarray(inputs["Wf1"], np.float32)
    bf1 = np.asarray(inputs["bf1"], np.float32)
    Wf2 = np.asarray(inputs["Wf2"], np.float32)
    bf2 = np.asarray(inputs["bf2"], np.float32)
    Wf3 = np.asarray(inputs["Wf3"], np.float32)
    bf3 = np.asarray(inputs["bf3"], np.float32)
    Wf4 = np.asarray(inputs["Wf4"], np.float32)
    bf4 = np.asarray(inputs["bf4"], np.float32)

    W1d, w1e = W1[:, :SD], W1[:, SD]
    A = (s @ W1d.T).astype(np.float32)                     # (N, 64)
    cdiag = (W2 @ np.maximum(b1 + w1e, 0)).astype(np.float32)

    shared = {
        "atneg": np.ascontiguousarray((-A.T)).astype(BF16),
        "l65": np.concatenate(
            [W2.T.astype(BF16), np.full((1, 128), -1.0, BF16)], 0),
        "cdiag": cdiag[:, None],
        "b2c": b2[:, None],
        "w1at": np.ascontiguousarray(Wf1[:, :128].T),
        "w1bt": np.ascontiguousarray(Wf1[:, 128:].T),
        "bf1c": bf1[:, None],
        "w2t": np.ascontiguousarray(Wf2.T),
        "bf2c": bf2[:, None],
        "w3t": np.ascontiguousarray(Wf3.T),
        "bf3c": bf3[:, None],
        "w4t": np.ascontiguousarray(Wf4.T),
        "bf4c": bf4[:, None],
        "ident": np.eye(SD, dtype=np.float32),
    }
    shared = {k: np.ascontiguousarray(v) for k, v in shared.items()}

    in_maps = []
    for c in range(NCORES):
        I = np.arange(c * RPC, (c + 1) * RPC)
        # fp32 mask with the reference's exact op order:
        # diff, square, ((d0+d1)+d2), compare with 4.0
        d = s[I, None, :3] - s[None, :, :3]
        sq = d * d
        d2 = (sq[..., 0] + sq[..., 1]) + sq[..., 2]
        pen = np.where(d2 >= OBS_R2, BIG, np.float32(0)).astype(np.float32)
        pen[np.arange(RPC), I] = BIG                       # exclude diagonal
        m = dict(shared)
        m["biasc"] = np.ascontiguousarray((A[I] + b1).T)
        m["pen"] = pen.astype(BF16)
        m["dst"] = np.ascontiguousarray((s[I] - s_ref[I]).T)
        m["ureft"] = np.ascontiguousarray(((s_ref[I] - s[I]) / np.float32(0.1)).T)
        in_maps.append(m)
    return in_maps


_CACHED = {}


def _axon_reset():
    try:
        import ctypes
        import jax
        jax.devices()
        lib = ctypes.CDLL("/opt/axon/libaxon_pjrt.so")
        lib.axon_reset.restype = ctypes.c_int64
        rc = lib.axon_reset()
        print(f"axon_reset rc={rc}", file=sys.stderr)
        return rc == 0
    except Exception as e:  # pragma: no cover
        print(f"axon_reset failed: {e}", file=sys.stderr)
        return False


def run(inputs, trace=False, tmpdir=None, h1_engine="gpsimd"):
    from concourse.bass_utils import run_bass_kernel_spmd
    key = h1_engine
    if key not in _CACHED:
        _CACHED[key] = build_module(h1_engine=h1_engine)
    nc = _CACHED[key]
    in_maps = prepare_in_maps(inputs)
    try:
        res = run_bass_kernel_spmd(
            nc, in_maps, core_ids=list(range(NCORES)), trace=trace, tmpdir=tmpdir)
    except Exception as e:
        if "unrecoverable" not in str(e).lower():
            raise
        if not _axon_reset():
            raise
        res = run_bass_kernel_spmd(
            nc, in_maps, core_ids=list(range(NCORES)), trace=trace, tmpdir=tmpdir)
    out = np.concatenate(
        [np.asarray(res.results[c]["yout"], np.float32) for c in range(NCORES)], 0)
    return out, res


def kernel(**inputs):
    out, _ = run(inputs, trace=False)
    return out


# revision 10
# speedup vs baseline: 10.8034x; 10.8034x over previous
"""Trainium2 Bass kernel for the pairwise message-passing network action model.

Full (unsharded) inputs in, full output out. Internally shards agents (rows i)
across 8 NeuronCores, data-parallel, per the row-sharding strategy:
each core computes a 128 x 1024 block of the pairwise interaction, does the
masked max-pool locally, and runs the replicated MLP head on its 128 agents.

Math restructure (exactly equivalent to the reference up to fp rounding):
  conv1: h1_ij = relu(W1d s_i - W1d s_j + b1 + delta_ij w1e)
       = relu(A_i - A_j + b1)  with A = s @ W1d.T   (diagonal handled separately)
  mask:  pooled_i = max_j mask_ij * relu(W2 h1_ij + b2)
       = relu(b2 + max_j (W2 h1_ij - pen_ij))  with pen in {0, BIG}
    The penalty row is appended as a 65th contraction row of the second
    matmul (lhsT row 64 = -1), so masking is free on the PE.
  diag:  z_ii = W2 relu(b1 + w1e) is a constant vector; the diagonal is
    excluded via pen and max'd back in at the end.
The mask itself (dist < 2) is computed on the host in fp32 with the same
operation order as the reference, so threshold decisions match bit-exactly.
"""

import os
import sys
import time
import types

import numpy as np
import ml_dtypes

import concourse.bass as bass
import concourse.tile as tile
from concourse import mybir
from concourse.vector_clock import ScopedClock

N = 1024
SD = 10
NCORES = 8
RPC = N // NCORES  # rows (agents) per core
BIG = np.float32(4096.0)
OBS_R2 = np.float32(4.0)  # OBS_RADIUS ** 2
BF16 = ml_dtypes.bfloat16

F32 = mybir.dt.float32
BF = mybir.dt.bfloat16


def _patch_tile_drain():
    """This walrus build rejects >1 sync-wait on CTRL (Drain/Nop) instructions;
    split the Tile exit drain's waits across single-wait nops."""
    if getattr(tile.TileContext, "_drain_split_patched", False):
        return

    def _drain_and_barrier_split(self, tick_clock, wait_clock):
        nc = self.nc
        drain_inst = nc.sync.drain()
        wait_clock.add_sem_waits(
            drain_inst.ins, ScopedClock({None: tick_clock.global_clock})
        )
        si = drain_inst.ins.sync_info
        waits = list(si.on_wait) if si and si.on_wait else []
        if len(waits) > 1:
            si.on_wait = [waits[0]]
            for w in waits[1:]:
                nop = nc.sync.nop(nofuse=True)
                nsi = nop.ins.sync_info
                if nsi is None:
                    nop.ins.sync_info = mybir.SyncInfo(on_wait=[w], on_update=[])
                else:
                    nsi.on_wait = [w]
        nc.all_engine_barrier()
        assert self.sems is not None
        popped = nc._tile_sem_poison_stack.pop()
        assert popped is self._sem_poison
        nc.clear_and_free_semaphores(list(self.sems.allocated().values()))
        nc.all_engine_barrier()

    tile.TileContext._drain_and_barrier = _drain_and_barrier_split

    # The Pool-engine (CoreV2) and CTRL instruction encodings in this build
    # accept only one embedded sync-wait. Hoist extra waits onto same-engine
    # NOPs right before the instruction (engines dispatch in order, so the
    # semantics are unchanged).
    _orig_postorder = tile.postorder_instruction_blocks
    _ws_counter = [0]

    def _needs_split(ins):
        return True

    def _postorder_with_wait_split(obib, start_bb, postordered):
        for bb, insts in obib.items():
            out = []
            changed = False
            for ins in insts:
                si = ins.sync_info
                waits = list(si.on_wait) if si and si.on_wait else []
                if len(waits) > 1 and _needs_split(ins):
                    changed = True
                    for w in waits[1:]:
                        _ws_counter[0] += 1
                        nop = mybir.InstNoOp(
                            name=f"WSPLIT-{_ws_counter[0]}", ins=[], outs=[])
                        nop.engine = ins.engine
                        nop.sync_info = mybir.SyncInfo(on_wait=[w], on_update=[])
                        out.append(nop)
                    si.on_wait = [waits[0]]
                out.append(ins)
            if changed:
                insts[:] = out
        return _orig_postorder(obib, start_bb, postordered)

    tile.postorder_instruction_blocks = _postorder_with_wait_split
    tile.TileContext._drain_split_patched = True


# ---------------------------------------------------------------------------
# device module
# ---------------------------------------------------------------------------

_INPUT_SPECS = [
    # name, shape, dtype
    ("atneg", (64, N), BF),        # -A.T, shared across cores
    ("biasc", (64, RPC), F32),     # (A[I] + b1).T, per core
    ("pen", (RPC, N), BF),         # penalty rows (0 / BIG), per core, diag=BIG
    ("l65", (65, 128), BF),        # [W2.T ; -1s] stationary operand
    ("cdiag", (128, 1), F32),      # W2 @ relu(b1 + w1e)
    ("b2c", (128, 1), F32),
    ("w1at", (128, 64), F32),      # Wf1[:, :128].T
    ("w1bt", (SD, 64), F32),       # Wf1[:, 128:].T
    ("bf1c", (64, 1), F32),
    ("w2t", (64, 128), F32),
    ("bf2c", (128, 1), F32),
    ("w3t", (128, 64), F32),
    ("bf3c", (64, 1), F32),
    ("w4t", (64, SD), F32),
    ("bf4c", (SD, 1), F32),
    ("dst", (SD, RPC), F32),       # (s - s_ref).T block
    ("ureft", (SD, RPC), F32),     # ((s_ref - s)/dt).T block
    ("ident", (SD, SD), F32),
]


def _emit(tc, ins, outs, h1_engine="mixed", reduce_mode="ttr"):
    nc = tc.nc
    AF = mybir.ActivationFunctionType
    ALU = mybir.AluOpType

    with tc.tile_pool(name="consts", bufs=1) as consts, \
         tc.tile_pool(name="persist", bufs=1) as persist:
        ct = {}
        for name, shape, dt in _INPUT_SPECS:
            t = consts.tile(list(shape), dt, tag=name)
            nc.sync.dma_start(t[:], ins[name][:])
            ct[name] = t

        pooledT = persist.tile([128, RPC], F32, tag="pooledT")

        def h1_compute(h1t, i):
            if h1_engine == "mixed":
                eng = "scalar" if i % 3 == 1 else "vector"
            else:
                eng = h1_engine
            if eng == "scalar":
                # relu(1.0 * (-A_j) + (A_i + b1))
                nc.scalar.activation(
                    h1t[0:64, :], ct["atneg"][:], AF.Relu,
                    bias=ct["biasc"][:, i:i + 1], scale=1.0)
            elif eng == "vector":
                nc.vector.tensor_scalar(
                    h1t[0:64, :], ct["atneg"][:],
                    scalar1=ct["biasc"][:, i:i + 1], scalar2=0.0,
                    op0=ALU.add, op1=ALU.max)
            else:
                nc.gpsimd.tensor_scalar(
                    h1t[0:64, :], ct["atneg"][:],
                    scalar1=ct["biasc"][:, i:i + 1], scalar2=0.0,
                    op0=ALU.add, op1=ALU.max)

        GRP = 2
        with tc.tile_pool(name="h1", bufs=4) as h1p, \
             tc.tile_pool(name="ps", bufs=2, space="PSUM") as psp:
            for r in range(RPC // GRP):
                ps = psp.tile([128, GRP * N], F32, tag="ps")
                for k in range(GRP):
                    i = GRP * r + k
                    h1t = h1p.tile([65, N], BF, tag=f"h1_{k}")
                    h1_compute(h1t, i)
                    nc.sync.dma_start(h1t[64:65, :], ct["pen"][i:i + 1, :])
                    for b in range(N // 512):
                        nc.tensor.matmul(
                            ps[:, k * N + b * 512: k * N + (b + 1) * 512],
                            ct["l65"][:],
                            h1t[:, b * 512:(b + 1) * 512],
                            start=True, stop=True,
                        )
                if reduce_mode == "pool":
                    nc.vector.pool(
                        pooledT[:, GRP * r: GRP * (r + 1)],
                        ps[:].rearrange("p (g n) -> p g n", g=GRP),
                        func=mybir.PoolFunctionType.max)
                else:
                    nc.vector.tensor_reduce(
                        pooledT[:, GRP * r: GRP * (r + 1)],
                        ps[:].rearrange("p (g n) -> p g n", g=GRP),
                        axis=mybir.AxisListType.X, op=ALU.max)

        # pooled = relu(max(pooled, cdiag) + b2), then the MLP head
        with tc.tile_pool(name="hsb", bufs=2) as hsb, \
             tc.tile_pool(name="hps", bufs=1, space="PSUM") as hps:
            pmax = hsb.tile([128, RPC], F32, tag="pmax")
            nc.vector.tensor_scalar(
                pmax[:], pooledT[:],
                scalar1=ct["cdiag"][:], scalar2=ct["b2c"][:],
                op0=ALU.max, op1=ALU.add,
            )
            feat = hsb.tile([128, RPC], F32, tag="feat")
            nc.scalar.activation(feat[:], pmax[:], AF.Relu)

            ps1 = hps.tile([64, RPC], F32, tag="ps1")
            nc.tensor.matmul(ps1[:], ct["w1at"][:], feat[:], start=True, stop=False)
            nc.tensor.matmul(ps1[:], ct["w1bt"][:], ct["dst"][:], start=False, stop=True)
            y1 = hsb.tile([64, RPC], F32, tag="y1")
            nc.scalar.activation(y1[:], ps1[:], AF.Relu, bias=ct["bf1c"][:])

            ps2 = hps.tile([128, RPC], F32, tag="ps2")
            nc.tensor.matmul(ps2[:], ct["w2t"][:], y1[:], start=True, stop=True)
            y2 = hsb.tile([128, RPC], F32, tag="y2")
            nc.scalar.activation(y2[:], ps2[:], AF.Relu, bias=ct["bf2c"][:])

            ps3 = hps.tile([64, RPC], F32, tag="ps3")
            nc.tensor.matmul(ps3[:], ct["w3t"][:], y2[:], start=True, stop=True)
            y3 = hsb.tile([64, RPC], F32, tag="y3")
            nc.scalar.activation(y3[:], ps3[:], AF.Relu, bias=ct["bf3c"][:])

            ps4 = hps.tile([SD, RPC], F32, tag="ps4")
            nc.tensor.matmul(ps4[:], ct["w4t"][:], y3[:], start=True, stop=True)
            y4 = hsb.tile([SD, RPC], F32, tag="y4")
            nc.scalar.activation(y4[:], ps4[:], AF.Identity, bias=ct["bf4c"][:])
            outT = hsb.tile([SD, RPC], F32, tag="outT")
            nc.vector.tensor_add(outT[:], y4[:], ct["ureft"][:])

            # transpose (SD, RPC) -> (RPC, SD) on the PE, then store
            pst = hps.tile([RPC, SD], F32, tag="pst")
            nc.tensor.transpose(pst[:], outT[:], ct["ident"][:])
            osb = hsb.tile([RPC, SD], F32, tag="osb")
            nc.scalar.copy(osb[:], pst[:])
            nc.sync.dma_start(outs["yout"][:], osb[:])


def build_module(h1_engine="mixed", reduce_mode="ttr"):
    _patch_tile_drain()
    nc = bass.Bass("TRN2", target_bir_lowering=False, debug=False,
                   num_devices=NCORES)
    ins = {}
    for name, shape, dt in _INPUT_SPECS:
        ins[name] = nc.dram_tensor(name, shape, dt, kind="ExternalInput").ap()
    outs = {"yout": nc.dram_tensor("yout", (RPC, SD), F32,
                                   kind="ExternalOutput").ap()}
    with tile.TileContext(nc) as tc:
        _emit(tc, ins, outs, h1_engine=h1_engine, reduce_mode=reduce_mode)
    return nc


# ---------------------------------------------------------------------------
# host side
# ---------------------------------------------------------------------------

def prepare_in_maps(inputs):
    s = np.asarray(inputs["s"], np.float32)
    s_ref = np.asarray(inputs["s_ref"], np.float32)
    W1 = np.asarray(inputs["W1"], np.float32)
    b1 = np.asarray(inputs["b1"], np.float32)
    W2 = np.asarray(inputs["W2"], np.float32)
    b2 = np.asarray(inputs["b2"], np.float32)
    Wf1 = np.asarray(inputs["Wf1"], np.float32)
    bf1 = np.asarray(inputs["bf1"], np.float32)
    Wf2 = np.asarray(inputs["Wf2"], np.float32)
    bf2 = np.asarray(inputs["bf2"], np.float32)
    Wf3 = np.asarray(inputs["Wf3"], np.float32)
    bf3 = np.asarray(inputs["bf3"], np.float32)
    Wf4 = np.asarray(inputs["Wf4"], np.float32)
    bf4 = np.asarray(inputs["bf4"], np.float32)

    W1d, w1e = W1[:, :SD], W1[:, SD]
    A = (s @ W1d.T).astype(np.float32)                     # (N, 64)
    cdiag = (W2 @ np.maximum(b1 + w1e, 0)).astype(np.float32)

    shared = {
        "atneg": np.ascontiguousarray((-A.T)).astype(BF16),
        "l65": np.concatenate(
            [W2.T.astype(BF16), np.full((1, 128), -1.0, BF16)], 0),
        "cdiag": cdiag[:, None],
        "b2c": b2[:, None],
        "w1at": np.ascontiguousarray(Wf1[:, :128].T),
        "w1bt": np.ascontiguousarray(Wf1[:, 128:].T),
        "bf1c": bf1[:, None],
        "w2t": np.ascontiguousarray(Wf2.T),
        "bf2c": bf2[:, None],
        "w3t": np.ascontiguousarray(Wf3.T),
        "bf3c": bf3[:, None],
        "w4t": np.ascontiguousarray(Wf4.T),
        "bf4c": bf4[:, None],
        "ident": np.eye(SD, dtype=np.float32),
    }
    shared = {k: np.ascontiguousarray(v) for k, v in shared.items()}

    in_maps = []
    for c in range(NCORES):
        I = np.arange(c * RPC, (c + 1) * RPC)
        # fp32 mask with the reference's exact op order:
        # diff, square, ((d0+d1)+d2), compare with 4.0
        d = s[I, None, :3] - s[None, :, :3]
        sq = d * d
        d2 = (sq[..., 0] + sq[..., 1]) + sq[..., 2]
        pen = np.where(d2 >= OBS_R2, BIG, np.float32(0)).astype(np.float32)
        pen[np.arange(RPC), I] = BIG                       # exclude diagonal
        m = dict(shared)
        m["biasc"] = np.ascontiguousarray((A[I] + b1).T)
        m["pen"] = pen.astype(BF16)
        m["dst"] = np.ascontiguousarray((s[I] - s_ref[I]).T)
        m["ureft"] = np.ascontiguousarray(((s_ref[I] - s[I]) / np.float32(0.1)).T)
        in_maps.append(m)
    return in_maps


_CACHED = {}


def _axon_reset():
    try:
        import ctypes
        import jax
        jax.devices()
        lib = ctypes.CDLL("/opt/axon/libaxon_pjrt.so")
        lib.axon_reset.restype = ctypes.c_int64
        rc = lib.axon_reset()
        print(f"axon_reset rc={rc}", file=sys.stderr)
        return rc == 0
    except Exception as e:  # pragma: no cover
        print(f"axon_reset failed: {e}", file=sys.stderr)
        return False


def run(inputs, trace=False, tmpdir=None, h1_engine="mixed", reduce_mode="ttr"):
    from concourse.bass_utils import run_bass_kernel_spmd
    key = (h1_engine, reduce_mode)
    if key not in _CACHED:
        _CACHED[key] = build_module(h1_engine=h1_engine, reduce_mode=reduce_mode)
    nc = _CACHED[key]
    in_maps = prepare_in_maps(inputs)
    try:
        res = run_bass_kernel_spmd(
            nc, in_maps, core_ids=list(range(NCORES)), trace=trace, tmpdir=tmpdir)
    except Exception as e:
        if "unrecoverable" not in str(e).lower():
            raise
        if not _axon_reset():
            raise
        res = run_bass_kernel_spmd(
            nc, in_maps, core_ids=list(range(NCORES)), trace=trace, tmpdir=tmpdir)
    out = np.concatenate(
        [np.asarray(res.results[c]["yout"], np.float32) for c in range(NCORES)], 0)
    return out, res


def kernel(**inputs):
    out, _ = run(inputs, trace=False)
    return out
